# revision 3
# baseline (speedup 1.0000x reference)
"""Trainium2 Bass kernel for nn_ContextEncoder (GRU feature encoder + DenseGAT readout).

Contract: kernel(**inputs) takes the FULL unsharded inputs (numpy, as produced
by setup_inputs) and returns the FULL output [B, CD] float32.

Strategy: data-parallel over the batch axis B across 8 NeuronCores. Each core
processes 16 batches = 2048 (batch, node) rows:
  - feature pipeline (speed + turn-angle) on device
  - 127-step GRU (hidden 128) with bf16 matmuls and fp32 PSUM accumulation
  - dense-GAT readout reduced analytically to small matmuls (only node 0 of
    the attention output is needed, and the per-head linear map commutes with
    the attention-weighted sum).
"""

import sys

sys.path.insert(0, "/opt/trn_rl_repo")

import numpy as np
import ml_dtypes

import concourse.bass as bass
import concourse.bacc as bacc
import concourse.mybir as mybir
import concourse.tile as tile
from concourse.bass_utils import run_bass_kernel_spmd

F32 = mybir.dt.float32
BF16 = mybir.dt.bfloat16
AF = mybir.ActivationFunctionType
ALU = mybir.AluOpType
AX = mybir.AxisListType

N_CORES = 8
B, N, L, HID, CD, HEADS = 128, 128, 128, 128, 128, 4
T = L - 1  # 127 GRU steps
BC = B // N_CORES  # batches per core = 16
R = BC * N  # rows per core = 2048
EPS = 1e-6
NEG_SLOPE = 0.2

# Abramowitz & Stegun 4.4.45: arccos(x) ~= sqrt(1-x) * poly(x), 0<=x<=1,
# |err| <= 6.7e-5 rad.
AC0, AC1, AC2, AC3 = 1.5707288, -0.2121144, 0.0742610, -0.0187293

NSTREAM = 4
SC = R // NSTREAM  # 512 rows per stream chunk
PACK_PAIRS = True  # row-group pack ih/bias matmuls across stream pairs


def _build_program(repeats=1, t_steps=T, skip_gru=False, skip_gat=False):
    nc = bacc.Bacc("TRN2", target_bir_lowering=False, debug=False,
                   num_devices=N_CORES)

    # Per-core inputs (already sharded/laid out by the host wrapper).
    xr_d = nc.dram_tensor("xr", [R, 2 * L], F32, kind="ExternalInput")
    whhT_d = nc.dram_tensor("whhT", [HID, 3 * HID], BF16, kind="ExternalInput")
    # ih lhsT (rows bias/wv/wa) replicated at partition bases {0, 32} for
    # pair-wise row-group packing; bhh_n likewise at {0, 32}.
    wih_d = nc.dram_tensor("wih_aug", [35, 3 * HID], BF16, kind="ExternalInput")
    bhhn_d = nc.dram_tensor("bhh_n", [33, HID], BF16, kind="ExternalInput")
    ident_d = nc.dram_tensor("ident", [128, 128], BF16, kind="ExternalInput")
    uwd_d = nc.dram_tensor("uwd", [HID, 2 * HEADS], BF16, kind="ExternalInput")
    wgT_d = nc.dram_tensor("wgT", [HID, HEADS * CD], BF16, kind="ExternalInput")
    gbias_d = nc.dram_tensor("gbias", [1, CD], BF16, kind="ExternalInput")
    out_d = nc.dram_tensor("out", [BC, CD], F32, kind="ExternalOutput")

    NT = R // 128  # 16 row tiles
    with tile.TileContext(nc) as tc:
        with (
            tc.tile_pool(name="dram", bufs=1, space="DRAM") as dpool,
            tc.tile_pool(name="const", bufs=1) as cpool,
        ):
            f3 = dpool.tile([T, 3, R], BF16)  # per-step rhs rows (v, ang, 1)
            ident = cpool.tile([128, 128], BF16, tag="ident")
            nc.sync.dma_start(ident[:], ident_d.ap())
            ones = cpool.tile([1, R], BF16, tag="ones")
            nc.vector.memset(ones[:], 1.0)
            for _ in range(repeats):
                _build_features(nc, tc, xr_d, f3, NT, ident)
                if not skip_gru:
                    _build_gru_gat(nc, tc, f3, whhT_d, wih_d, bhhn_d, ident,
                                   ones, uwd_d, wgT_d, gbias_d, out_d,
                                   t_steps, skip_gat)

    nc.compile()
    return nc


def _build_features(nc, tc, xr_d, f3, NT, ident):
    """v[t] = |x[t+1]-x[t]|, ang[t] = arccos(clip(pv*v/((pv+eps)(v+eps)))).

    Layout: rows on partitions (16 tiles of 128), t on free (127).
    Ends by transposing to [t, row] and DMAing into f3 DRAM [T, 3, R].
    """
    xr = xr_d.ap()  # [R, 2L] flat, contiguous per row

    with (
        tc.tile_pool(name="feat_in", bufs=3) as fin,
        tc.tile_pool(name="feat_tmp", bufs=3) as ftmp,
        tc.tile_pool(name="feat_keep", bufs=1) as fkeep,
        tc.tile_pool(name="feat_ps", bufs=3, space="PSUM") as fps,
    ):
        v_all = fkeep.tile([128, NT * T], F32, tag="v_all")
        ang = fkeep.tile([128, NT * T], F32, tag="ang")

        for p in range(NT):
            xt = fin.tile([128, 2 * L], F32, tag="xt")
            nc.sync.dma_start(xt[:], xr[p * 128:(p + 1) * 128, :])
            xl = xt[:].rearrange("p (l c) -> p l c", c=2)
            dx = ftmp.tile([128, T], F32, tag="dx")
            dy = ftmp.tile([128, T], F32, tag="dy")
            nc.vector.tensor_tensor(dx[:], xl[:, 1:, 0], xl[:, :-1, 0],
                                    ALU.subtract)
            nc.vector.tensor_tensor(dy[:], xl[:, 1:, 1], xl[:, :-1, 1],
                                    ALU.subtract)
            ss = ftmp.tile([128, T], F32, tag="ss")
            nc.vector.tensor_tensor(ss[:], dx[:], dx[:], ALU.mult)
            dy2 = ftmp.tile([128, T], F32, tag="dy2")
            nc.vector.tensor_tensor(dy2[:], dy[:], dy[:], ALU.mult)
            nc.vector.tensor_tensor(ss[:], ss[:], dy2[:], ALU.add)
            nc.scalar.activation(v_all[:, p * T:(p + 1) * T], ss[:], AF.Sqrt)

        v3 = v_all[:].rearrange("p (q t) -> p q t", t=T)
        # pv = previous-step speed (first step repeats itself)
        pv = fkeep.tile([128, NT * T], F32, tag="pv")
        pv3 = pv[:].rearrange("p (q t) -> p q t", t=T)
        nc.vector.tensor_copy(pv3[:, :, 1:], v3[:, :, :-1])
        nc.vector.tensor_copy(pv3[:, :, 0:1], v3[:, :, 0:1])

        b1 = fkeep.tile([128, NT * T], F32, tag="b1")
        nc.vector.tensor_scalar_add(b1[:], v_all[:], EPS)
        a1 = fkeep.tile([128, NT * T], F32, tag="a1")
        nc.vector.tensor_scalar_add(a1[:], pv[:], EPS)
        den = fkeep.tile([128, NT * T], F32, tag="den")
        nc.vector.tensor_tensor(den[:], a1[:], b1[:], ALU.mult)
        rden = fkeep.tile([128, NT * T], F32, tag="rden")
        nc.vector.reciprocal(rden[:], den[:])
        cos = fkeep.tile([128, NT * T], F32, tag="cos")
        nc.vector.tensor_tensor(cos[:], pv[:], v_all[:], ALU.mult)
        nc.vector.tensor_tensor(cos[:], cos[:], rden[:], ALU.mult)
        nc.vector.tensor_scalar_min(cos[:], cos[:], 1.0)

        # ang = sqrt(1-cos) * ((AC3*cos + AC2)*cos + AC1)*cos + AC0)
        s1 = fkeep.tile([128, NT * T], F32, tag="s1")
        nc.scalar.activation(s1[:], cos[:], AF.Sqrt, bias=1.0, scale=-1.0)
        poly = fkeep.tile([128, NT * T], F32, tag="poly")
        nc.vector.tensor_scalar(poly[:], cos[:], AC3, AC2, ALU.mult, ALU.add)
        nc.vector.tensor_tensor(poly[:], poly[:], cos[:], ALU.mult)
        nc.vector.tensor_scalar_add(poly[:], poly[:], AC1)
        nc.vector.tensor_tensor(poly[:], poly[:], cos[:], ALU.mult)
        nc.vector.tensor_scalar_add(poly[:], poly[:], AC0)
        nc.vector.tensor_tensor(ang[:], poly[:], s1[:], ALU.mult)

        # Cast to bf16, transpose tile-by-tile to [t, row], DMA into f3.
        vbf = fkeep.tile([128, NT * T], BF16, tag="vbf")
        abf = fkeep.tile([128, NT * T], BF16, tag="abf")
        nc.vector.tensor_copy(vbf[:], v_all[:])
        nc.vector.tensor_copy(abf[:], ang[:])
        onesb = fkeep.tile([128, R], BF16, tag="onesb")
        nc.vector.memset(onesb[:], 1.0)

        vt = fkeep.tile([T, R], BF16, tag="vt")
        at = fkeep.tile([T, R], BF16, tag="at")
        for p in range(NT):
            for src, dst in ((vbf, vt), (abf, at)):
                ps = fps.tile([T, 128], BF16, tag="tp")
                nc.tensor.transpose(ps[:], src[:, p * T:(p + 1) * T],
                                    ident[:])
                nc.vector.tensor_copy(dst[:, p * 128:(p + 1) * 128], ps[:])

        nc.sync.dma_start(f3[:, 0, :], onesb[0:T, :])
        nc.sync.dma_start(f3[:, 1, :], vt[:])
        nc.sync.dma_start(f3[:, 2, :], at[:])


def _build_gru_gat(nc, tc, f3, whhT_d, wih_d, bhhn_d, ident, ones, uwd_d,
                   wgT_d, gbias_d, out_d, t_steps=T, skip_gat=False):
    with (
        tc.tile_pool(name="wpool", bufs=1) as wpool,
        tc.tile_pool(name="hpool", bufs=2) as hpool,
    ):
        whhT = wpool.tile([HID, 3 * HID], BF16, tag="whhT")
        nc.sync.dma_start(whhT[:], whhT_d.ap())
        wih = wpool.tile([35, 3 * HID], BF16, tag="wih")
        nc.sync.dma_start(wih[:], wih_d.ap())
        bhhn = wpool.tile([33, HID], BF16, tag="bhhn")
        nc.sync.dma_start(bhhn[:], bhhn_d.ap())

        h_final = _gru(nc, tc, f3, whhT, wih, bhhn, ident, ones, hpool,
                       t_steps)
        if not skip_gat:
            _gat(nc, tc, h_final, uwd_d, wgT_d, gbias_d, ident, ones, out_d)
        else:
            osb = wpool.tile([BC, CD], F32, tag="osb_dbg")
            nc.vector.tensor_copy(osb[:], h_final[0][0:BC, 0:CD])
            nc.sync.dma_start(out_d.ap(), osb[:])


def _gru(nc, tc, f3, whhT, wih, bhhn, ident, ones, hpool, t_steps=T):
    """GRU steps over h [128 hid, 2048 rows] bf16, 4 row-streams."""
    with (
        tc.tile_pool(name="fpool", bufs=6) as fpool,
        tc.tile_pool(name="gru_sb", bufs=2 * NSTREAM) as gsb,
        tc.tile_pool(name="ps_rz", bufs=2, space="PSUM") as ps_rz,
        tc.tile_pool(name="ps_nh", bufs=2, space="PSUM") as ps_nh,
        tc.tile_pool(name="ps_gx", bufs=2, space="PSUM") as ps_gx,
    ):
        hs = []
        for s in range(NSTREAM):
            h0 = hpool.tile([HID, SC], BF16, tag=f"h{s}")
            nc.vector.memset(h0[:], 0.0)
            hs.append(h0)

        TB = 4  # timesteps per f-block DMA
        ftb = None
        for t in range(t_steps):
            # f rows (1, v_t, a_t) at partition bases 0 and 32 so stream
            # pairs can run K<=3 matmuls in distinct PE row groups.
            if t % TB == 0:
                nb = min(TB, t_steps - t)
                ftb = fpool.tile([35, TB * R], BF16, tag="ft")
                src = f3[t:t + nb].rearrange("t k r -> k t r")
                d0 = ftb[0:3, 0:nb * R].rearrange("k (t r) -> k t r", r=R)
                d1 = ftb[32:35, 0:nb * R].rearrange("k (t r) -> k t r", r=R)
                nc.sync.dma_start(d0, src)
                nc.sync.dma_start(d1, src)
            toff = (t % TB) * R
            ft = ftb[:, toff:toff + R]
            for pair in range(NSTREAM // 2):
                ss = (2 * pair, 2 * pair + 1)
                sls = [slice(s * SC, (s + 1) * SC) for s in ss]
                przs, pnhs, pgxs = [], [], []
                # packed ih matmuls first: only depend on ft
                for i, s in enumerate(ss):
                    bp = 32 * i if PACK_PAIRS else 0
                    prz = ps_rz.tile([128, 2 * SC], F32, tag="prz")
                    pnh = ps_nh.tile([128, SC], F32, tag="pnh")
                    pgx = ps_gx.tile([128, SC], F32, tag="pgx")
                    przs.append(prz); pnhs.append(pnh); pgxs.append(pgx)
                    nc.tensor.matmul(prz[:, 0:SC], wih[bp:bp + 3, 0:128],
                                     ft[bp:bp + 3, sls[i]],
                                     start=True, stop=False)
                    nc.tensor.matmul(prz[:, SC:], wih[bp:bp + 3, 128:256],
                                     ft[bp:bp + 3, sls[i]],
                                     start=True, stop=False)
                    nc.tensor.matmul(pgx[:], wih[bp:bp + 3, 256:384],
                                     ft[bp:bp + 3, sls[i]],
                                     start=True, stop=False)
                    nc.tensor.matmul(pnh[:], bhhn[bp:bp + 1, :],
                                     ft[bp:bp + 1, sls[i]],
                                     start=True, stop=False)
                for i, s in enumerate(ss):
                    prz, pnh, pgx = przs[i], pnhs[i], pgxs[i]
                    h_old = hs[s]
                    nc.tensor.matmul(prz[:, 0:SC], whhT[:, 0:128], h_old[:],
                                     start=False, stop=True)
                    nc.tensor.matmul(prz[:, SC:], whhT[:, 128:256], h_old[:],
                                     start=False, stop=True)
                    nc.tensor.matmul(pnh[:], whhT[:, 256:384], h_old[:],
                                     start=False, stop=True)
                    rz = gsb.tile([128, 2 * SC], BF16, tag="rz")
                    nc.scalar.activation(rz[:], prz[:], AF.Sigmoid)
                    t2 = gsb.tile([128, SC], BF16, tag="t2")
                    nc.vector.tensor_tensor(t2[:], rz[:, 0:SC], pnh[:],
                                            ALU.mult)
                    # accumulate r*gh_n onto the input part, tanh from PSUM
                    nc.tensor.matmul(pgx[:], ident[:], t2[:],
                                     start=False, stop=True)
                    nn = gsb.tile([128, SC], BF16, tag="nn")
                    nc.scalar.activation(nn[:], pgx[:], AF.Tanh)

                    d = gsb.tile([128, SC], BF16, tag="d")
                    nc.vector.tensor_tensor(d[:], h_old[:], nn[:],
                                            ALU.subtract)
                    nc.vector.tensor_tensor(d[:], rz[:, SC:], d[:], ALU.mult)
                    h_new = hpool.tile([HID, SC], BF16, tag=f"h{s}")
                    nc.vector.tensor_tensor(h_new[:], nn[:], d[:], ALU.add)
                    hs[s] = h_new
            h = hs
        return hs


def _gat(nc, tc, hs, uwd_d, wgT_d, gbias_d, ident, ones, out_d):
    """Attention from node 0 over all nodes, per batch of 128 rows.

    hs: list of NSTREAM tiles [HID, SC]; stream s holds rows [s*SC,(s+1)*SC),
    i.e. batches [4s, 4s+4).
    """
    with tc.tile_pool(name="gat_sb", bufs=1) as gsb:
        uwd = gsb.tile([HID, 2 * HEADS], BF16, tag="uwd")
        nc.sync.dma_start(uwd[:], uwd_d.ap())
        wgT = gsb.tile([HID, HEADS * CD], BF16, tag="wgT")
        nc.sync.dma_start(wgT[:], wgT_d.ap())
        gbias = gsb.tile([1, CD], BF16, tag="gbias")
        nc.sync.dma_start(gbias[:], gbias_d.ap())

        e = gsb.tile([HEADS, R], F32, tag="e")
        with tc.tile_pool(name="gat_ps", bufs=1, space="PSUM") as gps:
            # ssd[h, row] = <xh_row, u_h> ; dsd[h, row] = <xh_row, w_h>
            ssd = gps.tile([HEADS, R], F32, tag="ssd")
            dsd = gps.tile([HEADS, R], F32, tag="dsd")
            for c in range(R // SC):
                cs = slice(c * SC, (c + 1) * SC)
                nc.tensor.matmul(ssd[:, cs], uwd[:, 0:HEADS], hs[c][:],
                                 start=True, stop=True)
                nc.tensor.matmul(dsd[:, cs], uwd[:, HEADS:2 * HEADS],
                                 hs[c][:], start=True, stop=True)
            dsb = gsb.tile([HEADS, R], F32, tag="dsb")
            nc.vector.tensor_copy(dsb[:], dsd[:])

            # e[h, b*128+j] = s[h,b*128+j] + d[h, b*128] (attention logits)
            # d at node 0 per block, broadcast along j via a stride-0 AP.
            d0 = dsb[:].rearrange("h (b j) -> h b j", j=N)[:, :, 0:1]
            d0b = bass.AP(d0.tensor, d0.offset, list(d0.ap)[:-1] + [[0, N]])
            nc.vector.tensor_tensor(
                e[:].rearrange("h (b j) -> h b j", j=N),
                ssd[:].rearrange("h (b j) -> h b j", j=N), d0b, ALU.add)
        lr = gsb.tile([HEADS, R], F32, tag="lr")
        nc.scalar.activation(lr[:], e[:], AF.Lrelu, alpha=NEG_SLOPE)
        p = gsb.tile([HEADS, R], BF16, tag="p")
        nc.scalar.activation(p[:], lr[:], AF.Exp)

        # softmax denominators per (head, batch)
        ssum = gsb.tile([HEADS, BC], F32, tag="ssum")
        nc.vector.tensor_reduce(ssum[:], p[:].rearrange("h (b j) -> h b j",
                                                        j=N), AX.X, ALU.add)
        srec = gsb.tile([HEADS, BC], F32, tag="srec")
        nc.vector.reciprocal(srec[:], ssum[:])
        palpha = gsb.tile([HEADS, R], BF16, tag="palpha")
        s0 = srec[:]
        s0b = bass.AP(s0.tensor, s0.offset, list(s0.ap) + [[0, N]])
        nc.vector.tensor_tensor(
            palpha[:].rearrange("h (b j) -> h b j", j=N),
            p[:].rearrange("h (b j) -> h b j", j=N), s0b, ALU.mult)

        # transpose alpha and h per batch; ctx[f, (b h)] = sum_j hT[j,f]*aT[j,h]
        with tc.tile_pool(name="gat_ps2", bufs=2, space="PSUM") as gps2:
            pt = gsb.tile([128, HEADS * BC], BF16, tag="pt")
            ht = gsb.tile([128, R], BF16, tag="ht")
            ctx = gps2.tile([128, HEADS * BC], F32, tag="ctx")
            for b in range(BC):
                bs = slice(b * N, (b + 1) * N)
                lbs = slice((b % 4) * N, (b % 4 + 1) * N)
                pps = gps2.tile([128, HEADS], BF16, tag="pps")
                nc.tensor.transpose(pps[:], palpha[:, bs],
                                    ident[0:HEADS, 0:HEADS])
                nc.vector.tensor_copy(pt[:, b * HEADS:(b + 1) * HEADS],
                                      pps[:])
                nc.sync.dma_start_transpose(ht[:, bs], hs[b // 4][:, lbs])
            for b in range(BC):
                bs = slice(b * N, (b + 1) * N)
                nc.tensor.matmul(ctx[:, b * HEADS:(b + 1) * HEADS],
                                 ht[:, bs],
                                 pt[:, b * HEADS:(b + 1) * HEADS],
                                 start=True, stop=True)
            ctxs = gsb.tile([128, HEADS * BC], BF16, tag="ctxs")
            nc.vector.tensor_copy(ctxs[:], ctx[:])

            # out[b, c] = sum_h (W_h/4) ctx_bh + bias
            op = gps2.tile([BC, CD], F32, tag="op")
            ctx4 = ctxs[:].rearrange("f (b h) -> f h b", h=HEADS)
            for hh in range(HEADS):
                nc.tensor.matmul(op[:], ctx4[:, hh, :],
                                 wgT[:, hh * CD:(hh + 1) * CD],
                                 start=(hh == 0), stop=False)
            nc.tensor.matmul(op[:], ones[:, 0:BC], gbias[:], start=False,
                             stop=True)
            osb = gsb.tile([BC, CD], F32, tag="osb")
            nc.vector.tensor_copy(osb[:], op[:])
            nc.sync.dma_start(out_d.ap(), osb[:])


_NC_CACHE = None


def _get_program():
    global _NC_CACHE
    if _NC_CACHE is None:
        _NC_CACHE = _build_program()
    return _NC_CACHE


def _prep_in_maps(x, gru_wih, gru_whh, gru_bih, gru_bhh, gat_w, gat_att_src,
                  gat_att_dst, gat_bias):
    x = np.asarray(x, np.float32)
    gru_wih = np.asarray(gru_wih, np.float32)
    gru_whh = np.asarray(gru_whh, np.float32)
    gru_bih = np.asarray(gru_bih, np.float32)
    gru_bhh = np.asarray(gru_bhh, np.float32)
    gat_w = np.asarray(gat_w, np.float32)
    gat_att_src = np.asarray(gat_att_src, np.float32)
    gat_att_dst = np.asarray(gat_att_dst, np.float32)
    gat_bias = np.asarray(gat_bias, np.float32)

    bf = ml_dtypes.bfloat16

    whhT = np.ascontiguousarray(gru_whh.T).astype(bf)  # [128, 384]
    # ih lhsT rows (bias, wv, wa) replicated at partition bases {0, 32};
    # bias = bih+bhh for r,z gates, bih only for n (bhh_n enters via r*gh_n).
    bias3 = gru_bih + gru_bhh
    bias3 = bias3.copy()
    bias3[2 * HID:] = gru_bih[2 * HID:]
    blk = np.stack([bias3, gru_wih[:, 0], gru_wih[:, 1]])  # [3, 384]
    wih_aug = np.zeros((35, 3 * HID), np.float32)
    wih_aug[0:3] = blk
    wih_aug[32:35] = blk
    wih_aug = wih_aug.astype(bf)
    bhh_n = np.zeros((33, HID), np.float32)
    bhh_n[0] = gru_bhh[2 * HID:]
    bhh_n[32] = gru_bhh[2 * HID:]
    bhh_n = bhh_n.astype(bf)
    ident = np.eye(128, dtype=np.float32).astype(bf)

    W = gat_w.reshape(HEADS, CD, CD)  # [h, c, f]
    u = np.einsum("hcf,hc->hf", W, gat_att_src)
    w = np.einsum("hcf,hc->hf", W, gat_att_dst)
    uwd = np.ascontiguousarray(np.concatenate([u, w], 0).T).astype(bf)
    # per-head lhsT [f, c] of W_h/HEADS, laid side by side -> [128, 512]
    wgT = np.ascontiguousarray(
        np.concatenate([(W[h] / HEADS).T for h in range(HEADS)], axis=1)
    ).astype(bf)
    gbias = gat_bias.reshape(1, CD).astype(bf)

    shared = dict(whhT=whhT, wih_aug=wih_aug, bhh_n=bhh_n, ident=ident,
                  uwd=uwd, wgT=wgT, gbias=gbias)
    in_maps = []
    for c in range(N_CORES):
        xc = x[c * BC:(c + 1) * BC].reshape(R, 2 * L)
        in_maps.append({"xr": np.ascontiguousarray(xc), **shared})
    return in_maps


def kernel(x, gru_wih, gru_whh, gru_bih, gru_bhh, gat_w, gat_att_src,
           gat_att_dst, gat_bias):
    in_maps = _prep_in_maps(x, gru_wih, gru_whh, gru_bih, gru_bhh, gat_w,
                            gat_att_src, gat_att_dst, gat_bias)
    nc = _get_program()
    res = run_bass_kernel_spmd(nc, in_maps, list(range(N_CORES)))
    out = np.concatenate([res.results[c]["out"] for c in range(N_CORES)], 0)
    return out.astype(np.float32)



# revision 45
# speedup vs baseline: 1.6969x; 1.6969x over previous
"""Trainium2 Bass kernel for nn_ContextEncoder (GRU feature encoder + DenseGAT readout).

Contract: kernel(**inputs) takes the FULL unsharded inputs (numpy, as produced
by setup_inputs) and returns the FULL output [B, CD] float32.

Strategy: data-parallel over the batch axis B across 8 NeuronCores. Each core
processes 16 batches = 2048 (batch, node) rows:
  - feature pipeline (speed + turn-angle) on device
  - 127-step GRU (hidden 128) with bf16 matmuls and fp32 PSUM accumulation
  - dense-GAT readout reduced analytically to small matmuls (only node 0 of
    the attention output is needed, and the per-head linear map commutes with
    the attention-weighted sum).
"""

import sys

sys.path.insert(0, "/opt/trn_rl_repo")

import numpy as np
import ml_dtypes

import concourse.bass as bass
import concourse.bacc as bacc
import concourse.mybir as mybir
import concourse.tile as tile
from concourse.bass_utils import run_bass_kernel_spmd

F32 = mybir.dt.float32
BF16 = mybir.dt.bfloat16
AF = mybir.ActivationFunctionType
ALU = mybir.AluOpType
AX = mybir.AxisListType

N_CORES = 8
B, N, L, HID, CD, HEADS = 128, 128, 128, 128, 128, 4
T = L - 1  # 127 GRU steps
BC = B // N_CORES  # batches per core = 16
R = BC * N  # rows per core = 2048
EPS = 1e-6
NEG_SLOPE = 0.2

# Abramowitz & Stegun 4.4.45: arccos(x) ~= sqrt(1-x) * poly(x), 0<=x<=1,
# |err| <= 6.7e-5 rad.
AC0, AC1, AC2, AC3 = 1.5707288, -0.2121144, 0.0742610, -0.0187293

NSTREAM = 4
SC = R // NSTREAM  # 512 rows per stream chunk
PACK_PAIRS = True  # row-group pack ih/bias matmuls across stream pairs
TB = 8  # timesteps per f-block DMA
GRU_MODE = "v2"  # set per-build via _build_program(mode=...)
TB_CUR = TB


def _build_program(repeats=1, t_steps=T, skip_gru=False, skip_gat=False,
                   mode="v5_ph", tb=16):
    global GRU_MODE, TB_CUR
    GRU_MODE, TB_CUR = mode, tb
    nc = bacc.Bacc("TRN2", target_bir_lowering=False, debug=False,
                   num_devices=N_CORES)

    # Per-core inputs (already sharded/laid out by the host wrapper).
    xr_d = nc.dram_tensor("xr", [R, 2 * L], F32, kind="ExternalInput")
    whhT_d = nc.dram_tensor("whhT", [HID, 3 * HID], BF16, kind="ExternalInput")
    # ih lhsT (rows bias/wv/wa) replicated at partition bases {0, 32} for
    # pair-wise row-group packing; bhh_n likewise at {0, 32}.
    if GRU_MODE in ("v3", "v3d"):
        wih_d = nc.dram_tensor("wih_aug4", [99, 3 * HID], BF16,
                               kind="ExternalInput")
        bhhn_d = nc.dram_tensor("bhh_n4", [97, HID], BF16,
                                kind="ExternalInput")
    elif GRU_MODE.startswith(("v4", "v5", "v7")):
        wih_d = nc.dram_tensor("wih_aug4", [99, 3 * HID], BF16,
                               kind="ExternalInput")
        bhhn_d = nc.dram_tensor("bhh_col", [HID, 1], BF16,
                                kind="ExternalInput")
    else:
        wih_d = nc.dram_tensor("wih_aug", [35, 3 * HID], BF16,
                               kind="ExternalInput")
        bhhn_d = nc.dram_tensor("bhh_n", [33, HID], BF16,
                                kind="ExternalInput")
    ident_d = nc.dram_tensor("ident", [128, 128], BF16, kind="ExternalInput")
    uwd_d = nc.dram_tensor("uwd", [HID, 2 * HEADS], BF16, kind="ExternalInput")
    wgT_d = nc.dram_tensor("wgT", [HID, HEADS * CD], BF16, kind="ExternalInput")
    gbias_d = nc.dram_tensor("gbias", [1, CD], BF16, kind="ExternalInput")
    out_d = nc.dram_tensor("out", [BC, CD], F32, kind="ExternalOutput")

    NT = R // 128  # 16 row tiles
    with tile.TileContext(nc) as tc:
        with (
            tc.tile_pool(name="dram", bufs=1, space="DRAM") as dpool,
            tc.tile_pool(name="const", bufs=1) as cpool,
        ):
            f3 = dpool.tile([3, T, R], BF16)  # per-step rhs rows (1, v, ang)
            ident = cpool.tile([128, 128], BF16, tag="ident")
            nc.sync.dma_start(ident[:], ident_d.ap())
            ones = cpool.tile([1, R], BF16, tag="ones")
            nc.vector.memset(ones[:], 1.0)
            for _ in range(repeats):
                _build_features(nc, tc, xr_d, f3, NT, ident)
                if not skip_gru:
                    _build_gru_gat(nc, tc, f3, whhT_d, wih_d, bhhn_d, ident,
                                   ones, uwd_d, wgT_d, gbias_d, out_d,
                                   t_steps, skip_gat)

    nc.compile()
    return nc


def _build_features(nc, tc, xr_d, f3, NT, ident):
    """v[t] = |x[t+1]-x[t]|, ang[t] = arccos(clip(pv*v/((pv+eps)(v+eps)))).

    Layout: rows on partitions (16 tiles of 128), t on free (127).
    Ends by transposing to [t, row] and DMAing into f3 DRAM [T, 3, R].
    """
    xr = xr_d.ap()  # [R, 2L] flat, contiguous per row

    with (
        tc.tile_pool(name="feat_in", bufs=3) as fin,
        tc.tile_pool(name="feat_tmp", bufs=3) as ftmp,
        tc.tile_pool(name="feat_keep", bufs=1) as fkeep,
        tc.tile_pool(name="feat_ps", bufs=3, space="PSUM") as fps,
    ):
        v_all = fkeep.tile([128, NT * T], F32, tag="v_all")
        ang = fkeep.tile([128, NT * T], F32, tag="ang")

        for p in range(NT):
            xt = fin.tile([128, 2 * L], F32, tag="xt")
            nc.sync.dma_start(xt[:], xr[p * 128:(p + 1) * 128, :])
            xl = xt[:].rearrange("p (l c) -> p l c", c=2)
            dx = ftmp.tile([128, T], F32, tag="dx")
            dy = ftmp.tile([128, T], F32, tag="dy")
            nc.vector.tensor_tensor(dx[:], xl[:, 1:, 0], xl[:, :-1, 0],
                                    ALU.subtract)
            nc.vector.tensor_tensor(dy[:], xl[:, 1:, 1], xl[:, :-1, 1],
                                    ALU.subtract)
            ss = ftmp.tile([128, T], F32, tag="ss")
            nc.vector.tensor_tensor(ss[:], dx[:], dx[:], ALU.mult)
            dy2 = ftmp.tile([128, T], F32, tag="dy2")
            nc.vector.tensor_tensor(dy2[:], dy[:], dy[:], ALU.mult)
            nc.vector.tensor_tensor(ss[:], ss[:], dy2[:], ALU.add)
            nc.scalar.activation(v_all[:, p * T:(p + 1) * T], ss[:], AF.Sqrt)

        v3 = v_all[:].rearrange("p (q t) -> p q t", t=T)
        # pv = previous-step speed (first step repeats itself)
        pv = fkeep.tile([128, NT * T], F32, tag="pv")
        pv3 = pv[:].rearrange("p (q t) -> p q t", t=T)
        nc.vector.tensor_copy(pv3[:, :, 1:], v3[:, :, :-1])
        nc.vector.tensor_copy(pv3[:, :, 0:1], v3[:, :, 0:1])

        b1 = fkeep.tile([128, NT * T], F32, tag="b1")
        nc.vector.tensor_scalar_add(b1[:], v_all[:], EPS)
        a1 = fkeep.tile([128, NT * T], F32, tag="a1")
        nc.vector.tensor_scalar_add(a1[:], pv[:], EPS)
        den = fkeep.tile([128, NT * T], F32, tag="den")
        nc.vector.tensor_tensor(den[:], a1[:], b1[:], ALU.mult)
        rden = fkeep.tile([128, NT * T], F32, tag="rden")
        nc.vector.reciprocal(rden[:], den[:])
        cos = fkeep.tile([128, NT * T], F32, tag="cos")
        nc.vector.tensor_tensor(cos[:], pv[:], v_all[:], ALU.mult)
        nc.vector.tensor_tensor(cos[:], cos[:], rden[:], ALU.mult)
        nc.vector.tensor_scalar_min(cos[:], cos[:], 1.0)

        # ang = sqrt(1-cos) * ((AC3*cos + AC2)*cos + AC1)*cos + AC0)
        s1 = fkeep.tile([128, NT * T], F32, tag="s1")
        nc.scalar.activation(s1[:], cos[:], AF.Sqrt, bias=1.0, scale=-1.0)
        poly = fkeep.tile([128, NT * T], F32, tag="poly")
        nc.vector.tensor_scalar(poly[:], cos[:], AC3, AC2, ALU.mult, ALU.add)
        nc.vector.tensor_tensor(poly[:], poly[:], cos[:], ALU.mult)
        nc.vector.tensor_scalar_add(poly[:], poly[:], AC1)
        nc.vector.tensor_tensor(poly[:], poly[:], cos[:], ALU.mult)
        nc.vector.tensor_scalar_add(poly[:], poly[:], AC0)
        nc.vector.tensor_tensor(ang[:], poly[:], s1[:], ALU.mult)

        # Cast to bf16, transpose tile-by-tile to [t, row], DMA into f3.
        vbf = fkeep.tile([128, NT * T], BF16, tag="vbf")
        abf = fkeep.tile([128, NT * T], BF16, tag="abf")
        nc.vector.tensor_copy(vbf[:], v_all[:])
        nc.vector.tensor_copy(abf[:], ang[:])
        onesb = fkeep.tile([128, R], BF16, tag="onesb")
        nc.vector.memset(onesb[:], 1.0)

        vt = fkeep.tile([T, R], BF16, tag="vt")
        at = fkeep.tile([T, R], BF16, tag="at")
        for p in range(NT):
            for src, dst in ((vbf, vt), (abf, at)):
                ps = fps.tile([T, 128], BF16, tag="tp")
                nc.tensor.transpose(ps[:], src[:, p * T:(p + 1) * T],
                                    ident[:])
                nc.vector.tensor_copy(dst[:, p * 128:(p + 1) * 128], ps[:])

        nc.sync.dma_start(f3[0], onesb[0:T, :])
        nc.sync.dma_start(f3[1], vt[:])
        nc.sync.dma_start(f3[2], at[:])


def _build_gru_gat(nc, tc, f3, whhT_d, wih_d, bhhn_d, ident, ones, uwd_d,
                   wgT_d, gbias_d, out_d, t_steps=T, skip_gat=False):
    with (
        tc.tile_pool(name="wpool", bufs=1) as wpool,
        tc.tile_pool(name="hpool", bufs=2) as hpool,
    ):
        whhT = wpool.tile([HID, 3 * HID], BF16, tag="whhT")
        nc.sync.dma_start(whhT[:], whhT_d.ap())
        if GRU_MODE.startswith(("v4", "v5", "v7")):
            wih = wpool.tile([67, 3 * HID], BF16, tag="wih")
            nc.sync.dma_start(wih[:], wih_d.ap()[0:67, :])
            bhhn = wpool.tile([HID, 1], BF16, tag="bhhn")
            nc.sync.dma_start(bhhn[:], bhhn_d.ap())
        else:
            nbase = 4 if GRU_MODE in ("v3", "v3d") else 2
            wih = wpool.tile([32 * (nbase - 1) + 3, 3 * HID], BF16,
                             tag="wih")
            nc.sync.dma_start(wih[:], wih_d.ap())
            bhhn = wpool.tile([32 * (nbase - 1) + 1, HID], BF16, tag="bhhn")
            nc.sync.dma_start(bhhn[:], bhhn_d.ap())

        h_final = _gru(nc, tc, f3, whhT, wih, bhhn, ident, ones, hpool,
                       t_steps)
        if not skip_gat:
            _gat(nc, tc, h_final, uwd_d, wgT_d, gbias_d, ident, ones, out_d)
        else:
            osb = wpool.tile([BC, CD], F32, tag="osb_dbg")
            nc.vector.tensor_copy(osb[:], h_final[0][0:BC, 0:CD])
            nc.sync.dma_start(out_d.ap(), osb[:])


def _gru(nc, tc, f3, whhT, wih, bhhn, ident, ones, hpool, t_steps=T):
    if GRU_MODE in ("v1", "v15"):
        return _gru_v1(nc, tc, f3, whhT, wih, bhhn, ident, ones, hpool,
                       t_steps, pool_d=(GRU_MODE == "v15"))
    if GRU_MODE in ("v3", "v3d"):
        return _gru_v3(nc, tc, f3, whhT, wih, bhhn, ident, ones, hpool,
                       t_steps, pool_d=(GRU_MODE == "v3"))
    if GRU_MODE.startswith("v4"):
        return _gru_v4(nc, tc, f3, whhT, wih, bhhn, ident, ones, hpool,
                       t_steps, ablate=GRU_MODE[2:])
    if GRU_MODE.startswith("v5"):
        return _gru_v5(nc, tc, f3, whhT, wih, bhhn, ident, ones, hpool,
                       t_steps, variant=GRU_MODE[2:])
    if GRU_MODE.startswith("v7"):
        return _gru_v7(nc, tc, f3, whhT, wih, bhhn, ident, ones, hpool,
                       t_steps, variant=GRU_MODE[2:])


def _gru_v7(nc, tc, f3, whhT, wih, bhhn, ident, ones, hpool, t_steps=T,
            variant=""):
    """v5_p with the n-gate path pair-merged: pnh/pgx become pair-level
    [128, 2*SC] PSUM tiles (bufs=1 each), so t2/t3/tanh run once per pair
    on [*, 1024]. PE n-gate matmuls are emitted late so the bufs=1 wait
    lands after the previous pair's reads.
    """
    with (
        tc.tile_pool(name="fpool", bufs=2) as fpool,
        tc.tile_pool(name="gru_sb", bufs=2) as gsb,
        tc.tile_pool(name="ps_rz", bufs=2, space="PSUM") as ps_rz,
        tc.tile_pool(name="ps_nh", bufs=1, space="PSUM") as ps_nh,
        tc.tile_pool(name="ps_gx", bufs=1, space="PSUM") as ps_gx,
    ):
        hp = []
        for p in range(2):
            h0 = hpool.tile([HID, 2 * SC], BF16, tag=f"hp{p}", name=f"hp{p}")
            nc.vector.memset(h0[:], 0.0)
            hp.append(h0)

        ftb = None
        for t in range(t_steps):
            if t % TB_CUR == 0:
                nb = min(TB_CUR, t_steps - t)
                ftb = fpool.tile([67, TB_CUR * R], BF16, tag="ft", name="ft")
                src = f3[:, t:t + nb, :].rearrange("k t r -> k (t r)")
                nc.sync.dma_start(ftb[0:3, 0:nb * R], src)
                nc.sync.dma_start(ftb[32:35, 0:nb * R], src)
                nc.sync.dma_start(ftb[64:67, 0:nb * R], src)
            toff = (t % TB_CUR) * R
            ft = ftb[:, toff:toff + R]
            for pair in range(NSTREAM // 2):
                ss = (2 * pair, 2 * pair + 1)
                sls = [slice(s * SC, (s + 1) * SC) for s in ss]
                h_old = hp[pair]
                rzp = gsb.tile([128, 4 * SC], BF16, tag=f"rz{pair}",
                               name=f"rz{pair}")
                rz4 = rzp[:].rearrange("p (a s c) -> p a s c", a=2, s=2)
                nnp = gsb.tile([128, 2 * SC], BF16, tag=f"nn{pair}",
                               name=f"nn{pair}")
                przs = []
                for i, s in enumerate(ss):
                    prz = ps_rz.tile([128, 2 * SC], F32, tag="prz",
                                     name="prz")
                    przs.append(prz)
                    nc.tensor.matmul(prz[:, 0:SC], wih[0:3, 0:128],
                                     ft[0:3, sls[i]],
                                     start=True, stop=False,
                                     tile_position=(0, 0))
                    nc.tensor.matmul(prz[:, SC:], wih[32:35, 128:256],
                                     ft[32:35, sls[i]],
                                     start=True, stop=False,
                                     tile_position=(32, 0))
                for i, s in enumerate(ss):
                    hsl = h_old[:, i * SC:(i + 1) * SC]
                    nc.tensor.matmul(przs[i][:, 0:SC], whhT[:, 0:128], hsl,
                                     start=False, stop=True)
                    nc.tensor.matmul(przs[i][:, SC:], whhT[:, 128:256], hsl,
                                     start=False, stop=True)
                for i, s in enumerate(ss):
                    nc.scalar.activation(rz4[:, :, i, :], przs[i][:],
                                         AF.Sigmoid)
                # n-gate pair tiles (bufs=1) filled late
                pnhp = ps_nh.tile([128, 2 * SC], F32, tag="pnhp",
                                  name="pnhp")
                pgxp = ps_gx.tile([128, 2 * SC], F32, tag="pgxp",
                                  name="pgxp")
                for i, s in enumerate(ss):
                    nc.tensor.matmul(pgxp[:, i * SC:(i + 1) * SC],
                                     wih[64:67, 256:384],
                                     ft[64:67, sls[i]],
                                     start=True, stop=True,
                                     tile_position=(64, 0))
                    nc.tensor.matmul(pnhp[:, i * SC:(i + 1) * SC],
                                     whhT[:, 256:384],
                                     h_old[:, i * SC:(i + 1) * SC],
                                     start=True, stop=True)
                t2 = gsb.tile([128, 2 * SC], BF16, tag="t2", name="t2")
                nc.vector.scalar_tensor_tensor(
                    t2[:], pnhp[:], bhhn[:, 0:1], rzp[:, 0:2 * SC],
                    ALU.add, ALU.mult)
                t3 = gsb.tile([128, 2 * SC], BF16, tag="t3", name="t3")
                nc.vector.scalar_tensor_tensor(
                    t3[:], pgxp[:], 0.0, t2[:], ALU.bypass, ALU.add)
                nc.scalar.activation(nnp[:], t3[:], AF.Tanh)

                dp = gsb.tile([128, 2 * SC], BF16, tag=f"d{pair}",
                              name=f"d{pair}")
                nc.vector.tensor_tensor(dp[:], h_old[:], nnp[:],
                                        ALU.subtract)
                ep = gsb.tile([128, 2 * SC], BF16, tag=f"e{pair}",
                              name=f"e{pair}")
                nc.vector.tensor_tensor(ep[:], rzp[:, 2 * SC:], dp[:],
                                        ALU.mult)
                h_new = hpool.tile([HID, 2 * SC], BF16, tag=f"hp{pair}",
                                   name=f"hpn{pair}")
                nc.vector.tensor_tensor(h_new[:], nnp[:], ep[:], ALU.add)
                hp[pair] = h_new
        return hp
    return _gru_v2(nc, tc, f3, whhT, wih, bhhn, ident, ones, hpool, t_steps)


def _gru_v4(nc, tc, f3, whhT, wih, bhhn, ident, ones, hpool, t_steps=T,
            ablate=""):
    """v2 pair-merged update + gate-packed ih matmuls (row groups 0/32/64)
    + bhh_n folded into the t2 op via scalar_tensor_tensor:
        t2 = (gh_n + bhh_n) * r     [one DVE op, no K=1 bias matmul]
    ft replicated at partition bases {0,32,64} (3 DMAs per block).

    ablate: timing-ablation suffixes (values become wrong, timing valid):
      "_notanh"  skip tanh (nn := r half of rz)
      "_sighalf" sigmoid on r half only (z := r)
      "_noih"    skip ih matmuls (whh starts the psum groups)
      "_noupd"   h' := nn (skip d/e/h' elementwise update)
      "_noident" skip the ident accumulate matmul
    """
    with (
        tc.tile_pool(name="fpool", bufs=2) as fpool,
        tc.tile_pool(name="gru_sb", bufs=2) as gsb,
        tc.tile_pool(name="ps_rz", bufs=2, space="PSUM") as ps_rz,
        tc.tile_pool(name="ps_nh", bufs=2, space="PSUM") as ps_nh,
        tc.tile_pool(name="ps_gx", bufs=2, space="PSUM") as ps_gx,
    ):
        hp = []
        for p in range(2):
            h0 = hpool.tile([HID, 2 * SC], BF16, tag=f"hp{p}", name=f"hp{p}")
            nc.vector.memset(h0[:], 0.0)
            hp.append(h0)

        ftb = None
        for t in range(t_steps):
            if t % TB_CUR == 0:
                nb = min(TB_CUR, t_steps - t)
                ftb = fpool.tile([67, TB_CUR * R], BF16, tag="ft", name="ft")
                src = f3[:, t:t + nb, :].rearrange("k t r -> k (t r)")
                nc.sync.dma_start(ftb[0:3, 0:nb * R], src)
                nc.sync.dma_start(ftb[32:35, 0:nb * R], src)
                nc.sync.dma_start(ftb[64:67, 0:nb * R], src)
            toff = (t % TB_CUR) * R
            ft = ftb[:, toff:toff + R]
            for pair in range(NSTREAM // 2):
                ss = (2 * pair, 2 * pair + 1)
                sls = [slice(s * SC, (s + 1) * SC) for s in ss]
                h_old = hp[pair]
                rzp = gsb.tile([128, 4 * SC], BF16, tag=f"rz{pair}",
                               name=f"rz{pair}")
                rz4 = rzp[:].rearrange("p (a s c) -> p a s c", a=2, s=2)
                nnp = gsb.tile([128, 2 * SC], BF16, tag=f"nn{pair}",
                               name=f"nn{pair}")
                przs, pnhs, pgxs = [], [], []
                for i, s in enumerate(ss):
                    prz = ps_rz.tile([128, 2 * SC], F32, tag="prz",
                                     name="prz")
                    pnh = ps_nh.tile([128, SC], F32, tag="pnh", name="pnh")
                    pgx = ps_gx.tile([128, SC], F32, tag="pgx", name="pgx")
                    przs.append(prz); pnhs.append(pnh); pgxs.append(pgx)
                    if ablate != "_noih":
                        nc.tensor.matmul(prz[:, 0:SC], wih[0:3, 0:128],
                                         ft[0:3, sls[i]],
                                         start=True, stop=False,
                                         tile_position=(0, 0))
                        nc.tensor.matmul(prz[:, SC:], wih[32:35, 128:256],
                                         ft[32:35, sls[i]],
                                         start=True, stop=False,
                                         tile_position=(32, 0))
                        nc.tensor.matmul(pgx[:], wih[64:67, 256:384],
                                         ft[64:67, sls[i]],
                                         start=True, stop=False,
                                         tile_position=(64, 0))
                for i, s in enumerate(ss):
                    prz, pnh, pgx = przs[i], pnhs[i], pgxs[i]
                    ihs = ablate == "_noih"
                    hsl = h_old[:, i * SC:(i + 1) * SC]
                    nc.tensor.matmul(prz[:, 0:SC], whhT[:, 0:128], hsl,
                                     start=ihs, stop=True)
                    nc.tensor.matmul(prz[:, SC:], whhT[:, 128:256], hsl,
                                     start=ihs, stop=True)
                    nc.tensor.matmul(pnh[:], whhT[:, 256:384], hsl,
                                     start=True, stop=True)
                    # sigmoid: r -> rzp[:, i*SC], z -> rzp[:, 2*SC + i*SC]
                    if ablate == "_sighalf":
                        nc.scalar.activation(rzp[:, i * SC:(i + 1) * SC],
                                             prz[:, 0:SC], AF.Sigmoid)
                    else:
                        nc.scalar.activation(rz4[:, :, i, :], prz[:],
                                             AF.Sigmoid)
                    t2 = gsb.tile([128, SC], BF16, tag="t2", name="t2")
                    nc.vector.scalar_tensor_tensor(
                        t2[:], pnh[:], bhhn[:, 0:1],
                        rzp[:, i * SC:(i + 1) * SC], ALU.add, ALU.mult)
                    if ablate != "_noident":
                        nc.tensor.matmul(pgx[:], ident[:], t2[:],
                                         start=False, stop=True)
                    if ablate != "_notanh":
                        nc.scalar.activation(nnp[:, i * SC:(i + 1) * SC],
                                             pgx[:], AF.Tanh)

                nn_v = rzp[:, 0:2 * SC] if ablate == "_notanh" else nnp[:]
                z_v = (rzp[:, 0:2 * SC] if ablate == "_sighalf"
                       else rzp[:, 2 * SC:])
                if ablate == "_noupd":
                    hp[pair] = nnp
                    continue
                # pair-wide update: h' = nn + z*(h - nn)
                dp = gsb.tile([128, 2 * SC], BF16, tag=f"d{pair}",
                              name=f"d{pair}")
                nc.gpsimd.tensor_tensor(dp[:], h_old[:], nn_v, ALU.subtract)
                ep = gsb.tile([128, 2 * SC], BF16, tag=f"e{pair}",
                              name=f"e{pair}")
                nc.vector.tensor_tensor(ep[:], z_v, dp[:], ALU.mult)
                h_new = hpool.tile([HID, 2 * SC], BF16, tag=f"hp{pair}",
                                   name=f"hpn{pair}")
                nc.vector.tensor_tensor(h_new[:], nn_v, ep[:], ALU.add)
                hp[pair] = h_new
        return hp


def _gru_v3(nc, tc, f3, whhT, wih, bhhn, ident, ones, hpool, t_steps=T,
            pool_d=True):
    """v1 + 4-way row-group packing: each stream s runs its K<=3 ih/bias
    matmuls in PE row group 32*s, so all four streams' ih work overlaps.
    ft replicated at partition bases {0,32,64,96}; two replica DMAs go on
    the SWDGE (Pool) queue to keep the SP queue off the critical path.
    """
    with (
        tc.tile_pool(name="fpool", bufs=2) as fpool,
        tc.tile_pool(name="gru_sb", bufs=2 * NSTREAM) as gsb,
        tc.tile_pool(name="ps_rz", bufs=2, space="PSUM") as ps_rz,
        tc.tile_pool(name="ps_nh", bufs=2, space="PSUM") as ps_nh,
        tc.tile_pool(name="ps_gx", bufs=2, space="PSUM") as ps_gx,
    ):
        hs = []
        for s in range(NSTREAM):
            h0 = hpool.tile([HID, SC], BF16, tag=f"h{s}", name=f"h{s}")
            nc.vector.memset(h0[:], 0.0)
            hs.append(h0)

        ftb = None
        for t in range(t_steps):
            if t % TB_CUR == 0:
                nb = min(TB_CUR, t_steps - t)
                ftb = fpool.tile([99, TB_CUR * R], BF16, tag="ft", name="ft")
                src = f3[:, t:t + nb, :].rearrange("k t r -> k (t r)")
                nc.sync.dma_start(ftb[0:3, 0:nb * R], src)
                nc.sync.dma_start(ftb[32:35, 0:nb * R], src)
                nc.gpsimd.dma_start(ftb[64:67, 0:nb * R], src)
                nc.gpsimd.dma_start(ftb[96:99, 0:nb * R], src)
            toff = (t % TB_CUR) * R
            ft = ftb[:, toff:toff + R]
            for pair in range(NSTREAM // 2):
                ss = (2 * pair, 2 * pair + 1)
                sls = [slice(s * SC, (s + 1) * SC) for s in ss]
                przs, pnhs, pgxs = [], [], []
                for i, s in enumerate(ss):
                    # one row group per gate -> all 4 K<=3 matmuls of this
                    # stream run concurrently in distinct PE row groups
                    prz = ps_rz.tile([128, 2 * SC], F32, tag="prz",
                                     name="prz")
                    pnh = ps_nh.tile([128, SC], F32, tag="pnh", name="pnh")
                    pgx = ps_gx.tile([128, SC], F32, tag="pgx", name="pgx")
                    przs.append(prz); pnhs.append(pnh); pgxs.append(pgx)
                    nc.tensor.matmul(prz[:, 0:SC], wih[0:3, 0:128],
                                     ft[0:3, sls[i]],
                                     start=True, stop=False,
                                     tile_position=(0, 0))
                    nc.tensor.matmul(prz[:, SC:], wih[32:35, 128:256],
                                     ft[32:35, sls[i]],
                                     start=True, stop=False,
                                     tile_position=(32, 0))
                    nc.tensor.matmul(pgx[:], wih[64:67, 256:384],
                                     ft[64:67, sls[i]],
                                     start=True, stop=False,
                                     tile_position=(64, 0))
                    nc.tensor.matmul(pnh[:], bhhn[96:97, :],
                                     ft[96:97, sls[i]],
                                     start=True, stop=False,
                                     tile_position=(96, 0))
                for i, s in enumerate(ss):
                    prz, pnh, pgx = przs[i], pnhs[i], pgxs[i]
                    h_old = hs[s]
                    nc.tensor.matmul(prz[:, 0:SC], whhT[:, 0:128], h_old[:],
                                     start=False, stop=True)
                    nc.tensor.matmul(prz[:, SC:], whhT[:, 128:256], h_old[:],
                                     start=False, stop=True)
                    nc.tensor.matmul(pnh[:], whhT[:, 256:384], h_old[:],
                                     start=False, stop=True)
                    rz = gsb.tile([128, 2 * SC], BF16, tag="rz", name="rz")
                    nc.scalar.activation(rz[:], prz[:], AF.Sigmoid)
                    t2 = gsb.tile([128, SC], BF16, tag="t2", name="t2")
                    nc.vector.tensor_tensor(t2[:], rz[:, 0:SC], pnh[:],
                                            ALU.mult)
                    nc.tensor.matmul(pgx[:], ident[:], t2[:],
                                     start=False, stop=True)
                    nn = gsb.tile([128, SC], BF16, tag="nn", name="nn")
                    nc.scalar.activation(nn[:], pgx[:], AF.Tanh)

                    d = gsb.tile([128, SC], BF16, tag="d", name="d")
                    eng = nc.gpsimd if pool_d else nc.vector
                    eng.tensor_tensor(d[:], h_old[:], nn[:], ALU.subtract)
                    e = gsb.tile([128, SC], BF16, tag="e", name="e")
                    nc.vector.tensor_tensor(e[:], rz[:, SC:], d[:], ALU.mult)
                    h_new = hpool.tile([HID, SC], BF16, tag=f"h{s}",
                                       name=f"hn{s}")
                    nc.vector.tensor_tensor(h_new[:], nn[:], e[:], ALU.add)
                    hs[s] = h_new
        return hs


def _gru_v1(nc, tc, f3, whhT, wih, bhhn, ident, ones, hpool, t_steps=T,
            pool_d=False):
    """Baseline GRU: 4 independent row-streams, per-stream [*, SC] ops.

    pool_d: offload the (h - nn) subtract to the Pool engine.
    """
    with (
        tc.tile_pool(name="fpool", bufs=2) as fpool,
        tc.tile_pool(name="gru_sb", bufs=2 * NSTREAM) as gsb,
        tc.tile_pool(name="ps_rz", bufs=2, space="PSUM") as ps_rz,
        tc.tile_pool(name="ps_nh", bufs=2, space="PSUM") as ps_nh,
        tc.tile_pool(name="ps_gx", bufs=2, space="PSUM") as ps_gx,
    ):
        hs = []
        for s in range(NSTREAM):
            h0 = hpool.tile([HID, SC], BF16, tag=f"h{s}", name=f"h{s}")
            nc.vector.memset(h0[:], 0.0)
            hs.append(h0)

        ftb = None
        for t in range(t_steps):
            if t % TB_CUR == 0:
                nb = min(TB_CUR, t_steps - t)
                ftb = fpool.tile([35, TB_CUR * R], BF16, tag="ft", name="ft")
                src = f3[:, t:t + nb, :].rearrange("k t r -> k (t r)")
                nc.sync.dma_start(ftb[0:3, 0:nb * R], src)
                nc.sync.dma_start(ftb[32:35, 0:nb * R], src)
            toff = (t % TB_CUR) * R
            ft = ftb[:, toff:toff + R]
            for pair in range(NSTREAM // 2):
                ss = (2 * pair, 2 * pair + 1)
                sls = [slice(s * SC, (s + 1) * SC) for s in ss]
                przs, pnhs, pgxs = [], [], []
                for i, s in enumerate(ss):
                    bp = 32 * i if PACK_PAIRS else 0
                    prz = ps_rz.tile([128, 2 * SC], F32, tag="prz",
                                     name="prz")
                    pnh = ps_nh.tile([128, SC], F32, tag="pnh", name="pnh")
                    pgx = ps_gx.tile([128, SC], F32, tag="pgx", name="pgx")
                    przs.append(prz); pnhs.append(pnh); pgxs.append(pgx)
                    nc.tensor.matmul(prz[:, 0:SC], wih[bp:bp + 3, 0:128],
                                     ft[bp:bp + 3, sls[i]],
                                     start=True, stop=False)
                    nc.tensor.matmul(prz[:, SC:], wih[bp:bp + 3, 128:256],
                                     ft[bp:bp + 3, sls[i]],
                                     start=True, stop=False)
                    nc.tensor.matmul(pgx[:], wih[bp:bp + 3, 256:384],
                                     ft[bp:bp + 3, sls[i]],
                                     start=True, stop=False)
                    nc.tensor.matmul(pnh[:], bhhn[bp:bp + 1, :],
                                     ft[bp:bp + 1, sls[i]],
                                     start=True, stop=False)
                for i, s in enumerate(ss):
                    prz, pnh, pgx = przs[i], pnhs[i], pgxs[i]
                    h_old = hs[s]
                    nc.tensor.matmul(prz[:, 0:SC], whhT[:, 0:128], h_old[:],
                                     start=False, stop=True)
                    nc.tensor.matmul(prz[:, SC:], whhT[:, 128:256], h_old[:],
                                     start=False, stop=True)
                    nc.tensor.matmul(pnh[:], whhT[:, 256:384], h_old[:],
                                     start=False, stop=True)
                    rz = gsb.tile([128, 2 * SC], BF16, tag="rz", name="rz")
                    nc.scalar.activation(rz[:], prz[:], AF.Sigmoid)
                    t2 = gsb.tile([128, SC], BF16, tag="t2", name="t2")
                    nc.vector.tensor_tensor(t2[:], rz[:, 0:SC], pnh[:],
                                            ALU.mult)
                    nc.tensor.matmul(pgx[:], ident[:], t2[:],
                                     start=False, stop=True)
                    nn = gsb.tile([128, SC], BF16, tag="nn", name="nn")
                    nc.scalar.activation(nn[:], pgx[:], AF.Tanh)

                    d = gsb.tile([128, SC], BF16, tag="d", name="d")
                    eng = nc.gpsimd if pool_d else nc.vector
                    eng.tensor_tensor(d[:], h_old[:], nn[:], ALU.subtract)
                    e = gsb.tile([128, SC], BF16, tag="e", name="e")
                    nc.vector.tensor_tensor(e[:], rz[:, SC:], d[:], ALU.mult)
                    h_new = hpool.tile([HID, SC], BF16, tag=f"h{s}",
                                       name=f"hn{s}")
                    nc.vector.tensor_tensor(h_new[:], nn[:], e[:], ALU.add)
                    hs[s] = h_new
        return hs


def _gru_v5(nc, tc, f3, whhT, wih, bhhn, ident, ones, hpool, t_steps=T,
            variant=""):
    """v4 with the critical chain shortened:
      - no ident matmul: t3 = gx_n + t2 computed on DVE right after t2
        (same queue, no PE loop-back), tanh reads SBUF.
      - variant "":   update tail d,e,h' all on DVE (FIFO, 1 sem hop).
      - variant "_b": z-gate negated at load time so sigmoid yields
        zc = 1-z; then a = zc*h and c = h-a run on Pool DURING tanh
        (off-chain), leaving only b = zc*nn, h' = c+b on the chain.
    """
    with (
        tc.tile_pool(name="fpool", bufs=2) as fpool,
        tc.tile_pool(name="gru_sb", bufs=2) as gsb,
        tc.tile_pool(name="ps_rz", bufs=2, space="PSUM") as ps_rz,
        tc.tile_pool(name="ps_nh", bufs=2, space="PSUM") as ps_nh,
        tc.tile_pool(name="ps_gx", bufs=2, space="PSUM") as ps_gx,
    ):
        if "b" in variant:
            # negate z-gate weights once: sigmoid(z-cols) -> 1 - z
            nc.vector.tensor_scalar_mul(whhT[:, 128:256], whhT[:, 128:256],
                                        -1.0)
            nc.vector.tensor_scalar_mul(wih[:, 128:256], wih[:, 128:256],
                                        -1.0)
        hp = []
        for p in range(2):
            h0 = hpool.tile([HID, 2 * SC], BF16, tag=f"hp{p}", name=f"hp{p}")
            nc.vector.memset(h0[:], 0.0)
            hp.append(h0)

        ftb = None
        for t in range(t_steps):
            if t % TB_CUR == 0:
                nb = min(TB_CUR, t_steps - t)
                ftb = fpool.tile([67, TB_CUR * R], BF16, tag="ft", name="ft")
                src = f3[:, t:t + nb, :].rearrange("k t r -> k (t r)")
                nc.sync.dma_start(ftb[0:3, 0:nb * R], src)
                nc.sync.dma_start(ftb[32:35, 0:nb * R], src)
                nc.sync.dma_start(ftb[64:67, 0:nb * R], src)
            toff = (t % TB_CUR) * R
            ft = ftb[:, toff:toff + R]
            for pair in range(NSTREAM // 2):
                ss = (2 * pair, 2 * pair + 1)
                sls = [slice(s * SC, (s + 1) * SC) for s in ss]
                h_old = hp[pair]
                rzp = gsb.tile([128, 4 * SC], BF16, tag=f"rz{pair}",
                               name=f"rz{pair}")
                rz4 = rzp[:].rearrange("p (a s c) -> p a s c", a=2, s=2)
                nnp = gsb.tile([128, 2 * SC], BF16, tag=f"nn{pair}",
                               name=f"nn{pair}")
                przs, pnhs, pgxs = [], [], []
                for i, s in enumerate(ss):
                    prz = ps_rz.tile([128, 2 * SC], F32, tag="prz",
                                     name="prz")
                    pnh = ps_nh.tile([128, SC], F32, tag="pnh", name="pnh")
                    pgx = ps_gx.tile([128, SC], F32, tag="pgx", name="pgx")
                    przs.append(prz); pnhs.append(pnh); pgxs.append(pgx)
                    nc.tensor.matmul(prz[:, 0:SC], wih[0:3, 0:128],
                                     ft[0:3, sls[i]],
                                     start=True, stop=False,
                                     tile_position=(0, 0))
                    nc.tensor.matmul(prz[:, SC:], wih[32:35, 128:256],
                                     ft[32:35, sls[i]],
                                     start=True, stop=False,
                                     tile_position=(32, 0))
                    nc.tensor.matmul(pgx[:], wih[64:67, 256:384],
                                     ft[64:67, sls[i]],
                                     start=True, stop=True,
                                     tile_position=(64, 0))
                phased = "p" in variant
                t3s = []
                for i, s in enumerate(ss):
                    prz, pnh, pgx = przs[i], pnhs[i], pgxs[i]
                    hsl = h_old[:, i * SC:(i + 1) * SC]
                    nc.tensor.matmul(prz[:, 0:SC], whhT[:, 0:128], hsl,
                                     start=False, stop=True)
                    nc.tensor.matmul(prz[:, SC:], whhT[:, 128:256], hsl,
                                     start=False, stop=True)
                    nc.tensor.matmul(pnh[:], whhT[:, 256:384], hsl,
                                     start=True, stop=True)
                    if phased:
                        # sigmoid only; DVE/tanh phases emitted below so a
                        # stalled tanh never head-of-line-blocks the next
                        # stream's sigmoid in the ACT queue
                        nc.scalar.activation(rz4[:, :, i, :], prz[:],
                                             AF.Sigmoid)
                for i, s in enumerate(ss):
                    prz, pnh, pgx = przs[i], pnhs[i], pgxs[i]
                    if not phased:
                        nc.scalar.activation(rz4[:, :, i, :], prz[:],
                                             AF.Sigmoid)
                    t2 = gsb.tile([128, SC], BF16, tag="t2", name="t2")
                    t2eng = nc.gpsimd if "t" in variant else nc.vector
                    t2eng.scalar_tensor_tensor(
                        t2[:], pnh[:], bhhn[:, 0:1],
                        rzp[:, i * SC:(i + 1) * SC], ALU.add, ALU.mult)
                    t3 = gsb.tile([128, SC], BF16, tag="t3", name="t3")
                    nc.vector.scalar_tensor_tensor(
                        t3[:], pgx[:], 0.0, t2[:], ALU.bypass, ALU.add)
                    t3s.append(t3)
                    if not phased:
                        nc.scalar.activation(nnp[:, i * SC:(i + 1) * SC],
                                             t3[:], AF.Tanh)
                if phased:
                    for i, s in enumerate(ss):
                        nc.scalar.activation(nnp[:, i * SC:(i + 1) * SC],
                                             t3s[i][:], AF.Tanh)
                if "h" in variant:
                    # dummy weight loads, gated on this pair's sigmoid: PE
                    # activity during the ACT/DVE phase keeps the HAM clock
                    # gate from re-throttling the array between matmul bursts
                    nldw = 4 if "hh" in variant else 2
                    for q in range(nldw):
                        nc.tensor.ldweights(rzp[0:1, 128 * q:128 * (q + 1)])

                zsl = rzp[:, 2 * SC:]
                if "b" in variant:
                    # zc in z slots; a = zc*h and c = h - a off-chain on Pool
                    a = gsb.tile([128, 2 * SC], BF16, tag=f"a{pair}",
                                 name=f"a{pair}")
                    nc.gpsimd.tensor_tensor(a[:], zsl, h_old[:], ALU.mult)
                    c = gsb.tile([128, 2 * SC], BF16, tag=f"c{pair}",
                                 name=f"c{pair}")
                    nc.gpsimd.tensor_tensor(c[:], h_old[:], a[:],
                                            ALU.subtract)
                    b = gsb.tile([128, 2 * SC], BF16, tag=f"b{pair}",
                                 name=f"b{pair}")
                    nc.vector.tensor_tensor(b[:], zsl, nnp[:], ALU.mult)
                    h_new = hpool.tile([HID, 2 * SC], BF16, tag=f"hp{pair}",
                                       name=f"hpn{pair}")
                    nc.vector.tensor_tensor(h_new[:], c[:], b[:], ALU.add)
                else:
                    dp = gsb.tile([128, 2 * SC], BF16, tag=f"d{pair}",
                                  name=f"d{pair}")
                    deng = nc.gpsimd if "d" in variant else nc.vector
                    deng.tensor_tensor(dp[:], h_old[:], nnp[:], ALU.subtract)
                    ep = gsb.tile([128, 2 * SC], BF16, tag=f"e{pair}",
                                  name=f"e{pair}")
                    nc.vector.tensor_tensor(ep[:], zsl, dp[:], ALU.mult)
                    h_new = hpool.tile([HID, 2 * SC], BF16, tag=f"hp{pair}",
                                       name=f"hpn{pair}")
                    nc.vector.tensor_tensor(h_new[:], nnp[:], ep[:], ALU.add)
                hp[pair] = h_new
        return hp


def _gru_v2(nc, tc, f3, whhT, wih, bhhn, ident, ones, hpool, t_steps=T):
    """GRU steps over h [128 hid, 2048 rows] bf16.

    v2: pair-level h tiles [HID, 2*SC] (2 pairs x 1024 rows). Matmuls and
    activations stay per-stream (PSUM bank limit), but sigmoid writes into a
    shared pair tile laid out [r0 r1 | z0 z1] via a strided out-AP, so the
    h-update runs as three pair-wide [*, 1024] elementwise ops, with the
    subtract offloaded to the Pool (gpsimd) engine.
    """
    with (
        tc.tile_pool(name="fpool", bufs=2) as fpool,
        tc.tile_pool(name="gru_sb", bufs=2) as gsb,
        tc.tile_pool(name="ps_rz", bufs=2, space="PSUM") as ps_rz,
        tc.tile_pool(name="ps_nh", bufs=2, space="PSUM") as ps_nh,
        tc.tile_pool(name="ps_gx", bufs=2, space="PSUM") as ps_gx,
    ):
        hp = []
        for p in range(2):
            h0 = hpool.tile([HID, 2 * SC], BF16, tag=f"hp{p}")
            nc.vector.memset(h0[:], 0.0)
            hp.append(h0)

        ftb = None
        for t in range(t_steps):
            # f rows (1, v_t, a_t) at partition bases 0 and 32 so stream
            # pairs can run K<=3 matmuls in distinct PE row groups.
            if t % TB == 0:
                nb = min(TB, t_steps - t)
                ftb = fpool.tile([35, TB * R], BF16, tag="ft")
                src = f3[:, t:t + nb, :].rearrange("k t r -> k (t r)")
                nc.sync.dma_start(ftb[0:3, 0:nb * R], src)
                nc.sync.dma_start(ftb[32:35, 0:nb * R], src)
            toff = (t % TB) * R
            ft = ftb[:, toff:toff + R]
            for pair in range(NSTREAM // 2):
                ss = (2 * pair, 2 * pair + 1)
                sls = [slice(s * SC, (s + 1) * SC) for s in ss]
                h_old = hp[pair]
                rzp = gsb.tile([128, 4 * SC], BF16, tag=f"rz{pair}")
                rz4 = rzp[:].rearrange("p (a s c) -> p a s c", a=2, s=2)
                nnp = gsb.tile([128, 2 * SC], BF16, tag=f"nn{pair}")
                przs, pnhs, pgxs = [], [], []
                # packed ih matmuls first: only depend on ft
                for i, s in enumerate(ss):
                    bp = 32 * i if PACK_PAIRS else 0
                    prz = ps_rz.tile([128, 2 * SC], F32, tag="prz")
                    pnh = ps_nh.tile([128, SC], F32, tag="pnh")
                    pgx = ps_gx.tile([128, SC], F32, tag="pgx")
                    przs.append(prz); pnhs.append(pnh); pgxs.append(pgx)
                    nc.tensor.matmul(prz[:, 0:SC], wih[bp:bp + 3, 0:128],
                                     ft[bp:bp + 3, sls[i]],
                                     start=True, stop=False)
                    nc.tensor.matmul(prz[:, SC:], wih[bp:bp + 3, 128:256],
                                     ft[bp:bp + 3, sls[i]],
                                     start=True, stop=False)
                    nc.tensor.matmul(pgx[:], wih[bp:bp + 3, 256:384],
                                     ft[bp:bp + 3, sls[i]],
                                     start=True, stop=False)
                    nc.tensor.matmul(pnh[:], bhhn[bp:bp + 1, :],
                                     ft[bp:bp + 1, sls[i]],
                                     start=True, stop=False)
                for i, s in enumerate(ss):
                    prz, pnh, pgx = przs[i], pnhs[i], pgxs[i]
                    hsl = h_old[:, i * SC:(i + 1) * SC]
                    nc.tensor.matmul(prz[:, 0:SC], whhT[:, 0:128], hsl,
                                     start=False, stop=True)
                    nc.tensor.matmul(prz[:, SC:], whhT[:, 128:256], hsl,
                                     start=False, stop=True)
                    nc.tensor.matmul(pnh[:], whhT[:, 256:384], hsl,
                                     start=False, stop=True)
                    # sigmoid: r -> rzp[:, i*SC], z -> rzp[:, 2*SC + i*SC]
                    nc.scalar.activation(rz4[:, :, i, :], prz[:], AF.Sigmoid)
                    t2 = gsb.tile([128, SC], BF16, tag="t2")
                    nc.vector.tensor_tensor(t2[:], rzp[:, i * SC:(i + 1) * SC],
                                            pnh[:], ALU.mult)
                    # accumulate r*gh_n onto the input part, tanh from PSUM
                    nc.tensor.matmul(pgx[:], ident[:], t2[:],
                                     start=False, stop=True)
                    nc.scalar.activation(nnp[:, i * SC:(i + 1) * SC], pgx[:],
                                         AF.Tanh)

                # pair-wide update: h' = nn + z*(h - nn)
                dp = gsb.tile([128, 2 * SC], BF16, tag=f"d{pair}")
                nc.gpsimd.tensor_tensor(dp[:], h_old[:], nnp[:], ALU.subtract)
                ep = gsb.tile([128, 2 * SC], BF16, tag=f"e{pair}")
                nc.vector.tensor_tensor(ep[:], rzp[:, 2 * SC:], dp[:],
                                        ALU.mult)
                h_new = hpool.tile([HID, 2 * SC], BF16, tag=f"hp{pair}")
                nc.vector.tensor_tensor(h_new[:], nnp[:], ep[:], ALU.add)
                hp[pair] = h_new
        return hp


def _gat(nc, tc, hs, uwd_d, wgT_d, gbias_d, ident, ones, out_d):
    """Attention from node 0 over all nodes, per batch of 128 rows.

    hs: either 4 stream tiles [HID, SC] or 2 pair tiles [HID, 2*SC],
    covering rows in order.
    """

    def hsl(c):  # rows [c*SC, (c+1)*SC) as a [HID, SC] slice
        if len(hs) == 2:
            return hs[c // 2][:, (c % 2) * SC:(c % 2 + 1) * SC]
        return hs[c][:]
    with tc.tile_pool(name="gat_sb", bufs=1) as gsb:
        uwd = gsb.tile([HID, 2 * HEADS], BF16, tag="uwd")
        nc.sync.dma_start(uwd[:], uwd_d.ap())
        wgT = gsb.tile([HID, HEADS * CD], BF16, tag="wgT")
        nc.sync.dma_start(wgT[:], wgT_d.ap())
        gbias = gsb.tile([1, CD], BF16, tag="gbias")
        nc.sync.dma_start(gbias[:], gbias_d.ap())

        e = gsb.tile([HEADS, R], F32, tag="e")
        with tc.tile_pool(name="gat_ps", bufs=1, space="PSUM") as gps:
            # ssd[h, row] = <xh_row, u_h> ; dsd[h, row] = <xh_row, w_h>
            ssd = gps.tile([HEADS, R], F32, tag="ssd")
            dsd = gps.tile([HEADS, R], F32, tag="dsd")
            for c in range(R // SC):
                cs = slice(c * SC, (c + 1) * SC)
                nc.tensor.matmul(ssd[:, cs], uwd[:, 0:HEADS], hsl(c),
                                 start=True, stop=True)
                nc.tensor.matmul(dsd[:, cs], uwd[:, HEADS:2 * HEADS],
                                 hsl(c), start=True, stop=True)
            dsb = gsb.tile([HEADS, R], F32, tag="dsb")
            nc.vector.tensor_copy(dsb[:], dsd[:])

            # e[h, b*128+j] = s[h,b*128+j] + d[h, b*128] (attention logits)
            # d at node 0 per block, broadcast along j via a stride-0 AP.
            d0 = dsb[:].rearrange("h (b j) -> h b j", j=N)[:, :, 0:1]
            d0b = bass.AP(d0.tensor, d0.offset, list(d0.ap)[:-1] + [[0, N]])
            nc.vector.tensor_tensor(
                e[:].rearrange("h (b j) -> h b j", j=N),
                ssd[:].rearrange("h (b j) -> h b j", j=N), d0b, ALU.add)
        lr = gsb.tile([HEADS, R], F32, tag="lr")
        nc.scalar.activation(lr[:], e[:], AF.Lrelu, alpha=NEG_SLOPE)
        p = gsb.tile([HEADS, R], BF16, tag="p")
        nc.scalar.activation(p[:], lr[:], AF.Exp)

        # softmax denominators per (head, batch)
        ssum = gsb.tile([HEADS, BC], F32, tag="ssum")
        nc.vector.tensor_reduce(ssum[:], p[:].rearrange("h (b j) -> h b j",
                                                        j=N), AX.X, ALU.add)
        srec = gsb.tile([HEADS, BC], F32, tag="srec")
        nc.vector.reciprocal(srec[:], ssum[:])
        palpha = gsb.tile([HEADS, R], BF16, tag="palpha")
        s0 = srec[:]
        s0b = bass.AP(s0.tensor, s0.offset, list(s0.ap) + [[0, N]])
        nc.vector.tensor_tensor(
            palpha[:].rearrange("h (b j) -> h b j", j=N),
            p[:].rearrange("h (b j) -> h b j", j=N), s0b, ALU.mult)

        # transpose alpha and h per batch; ctx[f, (b h)] = sum_j hT[j,f]*aT[j,h]
        with tc.tile_pool(name="gat_ps2", bufs=2, space="PSUM") as gps2:
            pt = gsb.tile([128, HEADS * BC], BF16, tag="pt")
            ht = gsb.tile([128, R], BF16, tag="ht")
            ctx = gps2.tile([128, HEADS * BC], F32, tag="ctx")
            for b in range(BC):
                bs = slice(b * N, (b + 1) * N)
                pps = gps2.tile([128, HEADS], BF16, tag="pps")
                nc.tensor.transpose(pps[:], palpha[:, bs],
                                    ident[0:HEADS, 0:HEADS])
                nc.vector.tensor_copy(pt[:, b * HEADS:(b + 1) * HEADS],
                                      pps[:])
                nc.sync.dma_start_transpose(
                    ht[:, bs], hsl(b // 4)[:, (b % 4) * N:(b % 4 + 1) * N])
            for b in range(BC):
                bs = slice(b * N, (b + 1) * N)
                nc.tensor.matmul(ctx[:, b * HEADS:(b + 1) * HEADS],
                                 ht[:, bs],
                                 pt[:, b * HEADS:(b + 1) * HEADS],
                                 start=True, stop=True)
            ctxs = gsb.tile([128, HEADS * BC], BF16, tag="ctxs")
            nc.vector.tensor_copy(ctxs[:], ctx[:])

            # out[b, c] = sum_h (W_h/4) ctx_bh + bias
            op = gps2.tile([BC, CD], F32, tag="op")
            ctx4 = ctxs[:].rearrange("f (b h) -> f h b", h=HEADS)
            for hh in range(HEADS):
                nc.tensor.matmul(op[:], ctx4[:, hh, :],
                                 wgT[:, hh * CD:(hh + 1) * CD],
                                 start=(hh == 0), stop=False)
            nc.tensor.matmul(op[:], ones[:, 0:BC], gbias[:], start=False,
                             stop=True)
            osb = gsb.tile([BC, CD], F32, tag="osb")
            nc.vector.tensor_copy(osb[:], op[:])
            nc.sync.dma_start(out_d.ap(), osb[:])


_NC_CACHE = None


def _get_program():
    global _NC_CACHE
    if _NC_CACHE is None:
        _NC_CACHE = _build_program()
    return _NC_CACHE


def _prep_in_maps(x, gru_wih, gru_whh, gru_bih, gru_bhh, gat_w, gat_att_src,
                  gat_att_dst, gat_bias):
    x = np.asarray(x, np.float32)
    gru_wih = np.asarray(gru_wih, np.float32)
    gru_whh = np.asarray(gru_whh, np.float32)
    gru_bih = np.asarray(gru_bih, np.float32)
    gru_bhh = np.asarray(gru_bhh, np.float32)
    gat_w = np.asarray(gat_w, np.float32)
    gat_att_src = np.asarray(gat_att_src, np.float32)
    gat_att_dst = np.asarray(gat_att_dst, np.float32)
    gat_bias = np.asarray(gat_bias, np.float32)

    bf = ml_dtypes.bfloat16

    whhT = np.ascontiguousarray(gru_whh.T).astype(bf)  # [128, 384]
    # ih lhsT rows (bias, wv, wa) replicated at partition bases {0, 32};
    # bias = bih+bhh for r,z gates, bih only for n (bhh_n enters via r*gh_n).
    bias3 = gru_bih + gru_bhh
    bias3 = bias3.copy()
    bias3[2 * HID:] = gru_bih[2 * HID:]
    blk = np.stack([bias3, gru_wih[:, 0], gru_wih[:, 1]])  # [3, 384]
    wih_aug = np.zeros((35, 3 * HID), np.float32)
    wih_aug[0:3] = blk
    wih_aug[32:35] = blk
    wih_aug = wih_aug.astype(bf)
    bhh_n = np.zeros((33, HID), np.float32)
    bhh_n[0] = gru_bhh[2 * HID:]
    bhh_n[32] = gru_bhh[2 * HID:]
    bhh_n = bhh_n.astype(bf)
    wih_aug4 = np.zeros((99, 3 * HID), np.float32)
    bhh_n4 = np.zeros((97, HID), np.float32)
    for g in range(4):
        wih_aug4[32 * g:32 * g + 3] = blk
        bhh_n4[32 * g] = gru_bhh[2 * HID:]
    wih_aug4 = wih_aug4.astype(bf)
    bhh_n4 = bhh_n4.astype(bf)
    ident = np.eye(128, dtype=np.float32).astype(bf)

    W = gat_w.reshape(HEADS, CD, CD)  # [h, c, f]
    u = np.einsum("hcf,hc->hf", W, gat_att_src)
    w = np.einsum("hcf,hc->hf", W, gat_att_dst)
    uwd = np.ascontiguousarray(np.concatenate([u, w], 0).T).astype(bf)
    # per-head lhsT [f, c] of W_h/HEADS, laid side by side -> [128, 512]
    wgT = np.ascontiguousarray(
        np.concatenate([(W[h] / HEADS).T for h in range(HEADS)], axis=1)
    ).astype(bf)
    gbias = gat_bias.reshape(1, CD).astype(bf)

    bhh_col = np.ascontiguousarray(
        gru_bhh[2 * HID:].reshape(HID, 1)).astype(bf)
    shared = dict(whhT=whhT, wih_aug=wih_aug, bhh_n=bhh_n,
                  wih_aug4=wih_aug4, bhh_n4=bhh_n4, bhh_col=bhh_col,
                  ident=ident, uwd=uwd, wgT=wgT, gbias=gbias)
    in_maps = []
    for c in range(N_CORES):
        xc = x[c * BC:(c + 1) * BC].reshape(R, 2 * L)
        in_maps.append({"xr": np.ascontiguousarray(xc), **shared})
    return in_maps


def kernel(x, gru_wih, gru_whh, gru_bih, gru_bhh, gat_w, gat_att_src,
           gat_att_dst, gat_bias):
    in_maps = _prep_in_maps(x, gru_wih, gru_whh, gru_bih, gru_bhh, gat_w,
                            gat_att_src, gat_att_dst, gat_bias)
    nc = _get_program()
    res = run_bass_kernel_spmd(nc, in_maps, list(range(N_CORES)))
    out = np.concatenate([res.results[c]["out"] for c in range(N_CORES)], 0)
    return out.astype(np.float32)

